# revision 9
# baseline (speedup 1.0000x reference)
"""Multi-head causal attention with RoPE on 8 Trainium2 NeuronCores.

Sharding: tensor-parallel over heads x data-parallel over batch.
Core c handles batch b = c//4 and heads [4*(c%4), 4*(c%4)+4) (Hl=256 of Hd=1024).
Each core computes q/k/v projections for its head slice (column-split Wq/Wk/Wv),
RoPE, causal softmax attention, and a partial output projection (row-split Wo).
The host sums the 4 partial outputs per batch (the "all-reduce").

Device layouts (per core, S=2048, E=1024, Hl=256, D=64):
  xT   [E, S]    x transposed (host-side) so E rides the partition dim
  qT/kT slabs [128, S] x2: partitions = 2 heads x 64 dims, free = seq
  v    16 tiles [128, 260]: partitions = seq chunk, free = 4 heads x (64 dims + ones col)
  scores computed transposed (keys on partitions), softmax Z via ones-column of v,
  normalization by 1/Z broadcast via a DRAM-roundtrip partition-broadcast DMA.

All matmuls run in float32r (single-pass PE, ~1.5e-4 rel err measured on HW).
Attention processes the two heads of a slab in lockstep: their K=64 score
matmuls land on disjoint PE row groups (base partitions 0/64) and overlap,
and the interleave keeps the PE fed while ACT computes exp.
"""
import sys

sys.path.insert(0, "/opt/trn_rl_repo")
import numpy as np  # noqa: E402

N_HEADS = 16
B, S, E, HD = 2, 2048, 1024, 1024
D = HD // N_HEADS  # 64
HPC = 4            # heads per core
HL = HPC * D       # 256
NCORES = 8
ROPE_BASE = 10000.0

_built = None


def _build_nc():
    import concourse.bass as bass
    import concourse.tile as tile
    from concourse import bacc, mybir

    F32 = mybir.dt.float32
    F32R = mybir.dt.float32r
    Exp = mybir.ActivationFunctionType.Exp
    is_ge = mybir.AluOpType.is_ge
    ts = bass.ts

    nc = bacc.Bacc("TRN2", target_bir_lowering=False, debug=False)
    xT_d = nc.dram_tensor("xT", [E, S], F32, kind="ExternalInput").ap()
    wq_d = nc.dram_tensor("wq", [E, HL], F32, kind="ExternalInput").ap()
    wk_d = nc.dram_tensor("wk", [E, HL], F32, kind="ExternalInput").ap()
    wv_d = nc.dram_tensor("wv", [E, HL], F32, kind="ExternalInput").ap()
    wo_d = nc.dram_tensor("wo", [HL, E], F32, kind="ExternalInput").ap()
    cos_d = nc.dram_tensor("cosx", [128, S], F32, kind="ExternalInput").ap()
    sin_d = nc.dram_tensor("sinx", [128, S], F32, kind="ExternalInput").ap()
    out_d = nc.dram_tensor("out", [S, E], F32, kind="ExternalOutput").ap()
    zscr_d = nc.dram_tensor("zscr", [HPC, S], F32).ap()  # internal scratch

    ECH = E // 128   # 8 e-chunks
    SCH = S // 128   # 16 seq chunks
    SB = S // 512    # 4 seq blocks
    swap_mask = []
    for i in range(16):
        swap_mask += [2 * i + 1, 2 * i]

    with tile.TileContext(nc) as tc:
        with (
            tc.tile_pool(name="persist", bufs=1) as pp,
            tc.tile_pool(name="evict", bufs=3) as ev,
        ):
            # persistent tiles
            qT = [pp.tile([128, S], F32R, tag=f"qT{c}", name=f"qT{c}") for c in range(2)]
            kT = [pp.tile([128, S], F32R, tag=f"kT{c}", name=f"kT{c}") for c in range(2)]
            vt = [pp.tile([128, HPC * (D + 1)], F32R, tag=f"v{t}", name=f"v{t}")
                  for t in range(SCH)]
            oT = [pp.tile([128, S], F32R, tag=f"oT{c}", name=f"oT{c}") for c in range(2)]
            cosx = pp.tile([128, S], F32R, tag="cosx", name="cosx")
            sinx = pp.tile([128, S], F32R, tag="sinx", name="sinx")
            wo_t = pp.tile([128, 2, E], F32R, tag="wo", name="wo")

            # small/constant loads on the scalar queue, weights spread across
            # queues so nothing waits behind the big xT stream


            # ---------------- Phase B: projections + RoPE ----------------
            with (
                tc.tile_pool(name="bx", bufs=1) as bx,
                tc.tile_pool(name="bswp", bufs=2) as bswp,
                tc.tile_pool(name="bps", bufs=8, space="PSUM") as bps,
            ):
                wq_t = bx.tile([128, ECH, HL], F32R, tag="wq", name="wq")
                wk_t = bx.tile([128, ECH, HL], F32R, tag="wk", name="wk")
                wv_t = bx.tile([128, ECH, HL], F32R, tag="wv", name="wv")
                def wdma(eng, w_t_, w_d_):
                    eng.dma_start(
                        out=w_t_[:],
                        in_=w_d_.rearrange("(c p) m -> p c m", p=128).bitcast(F32R),
                    )
                xt = [bx.tile([128, S], F32R, tag=f"x{e}", name=f"x{e}")
                      for e in range(ECH)]

                def xdma(eng, e):
                    eng.dma_start(
                        out=xt[e][:], in_=xT_d[e * 128:(e + 1) * 128, :].bitcast(F32R)
                    )
                wdma(nc.sync, wq_t, wq_d)
                for e in range(0, ECH, 2):
                    xdma(nc.sync, e)
                wdma(nc.scalar, wk_t, wk_d)
                nc.scalar.dma_start(out=cosx[:], in_=cos_d.bitcast(F32R))
                nc.scalar.dma_start(out=sinx[:], in_=sin_d.bitcast(F32R))
                for e in range(1, ECH, 2):
                    xdma(nc.scalar, e)
                wdma(nc.scalar, wv_t, wv_d)
                nc.scalar.dma_start(
                    out=wo_t[:],
                    in_=wo_d.rearrange("(c p) e -> p c e", p=128).bitcast(F32R),
                )

                # q/k projections -> transposed slabs, RoPE interleaved so the
                # DVE rope work overlaps the next chunk's PE matmuls
                def rope(dest, c):
                    sw = bswp.tile([128, S], F32R, tag="swp", name="swp")
                    nc.vector.stream_shuffle(
                        out=sw[:].bitcast(F32),
                        in_=dest[c][:].bitcast(F32),
                        mask=swap_mask,
                    )
                    nc.vector.tensor_mul(out=sw[:], in0=sw[:], in1=sinx[:])
                    nc.vector.tensor_mul(out=dest[c][:], in0=dest[c][:], in1=cosx[:])
                    nc.vector.tensor_add(out=dest[c][:], in0=dest[c][:], in1=sw[:])

                for w_t_, dest in ((wq_t, qT), (wk_t, kT)):
                    for m in range(2):
                        for j in range(SB):
                            ps = bps.tile([128, 512], F32, tag="mm", name="mm")
                            for e in range(ECH):
                                nc.tensor.matmul(
                                    ps[:],
                                    w_t_[:, e, m * 128:(m + 1) * 128],
                                    xt[e][:, ts(j, 512)],
                                    start=(e == 0),
                                    stop=(e == ECH - 1),
                                )
                            nc.vector.tensor_copy(
                                out=dest[m][:, ts(j, 512)], in_=ps[:]
                            )
                        rope(dest, m)
                # v projection -> seq-partition tiles with ones column
                for t in range(SCH):
                    nc.gpsimd.memset(
                        vt[t].rearrange("p (h c) -> p h c", c=D + 1)[:, :, D:D + 1]
                        .bitcast(F32),
                        1.0,
                    )
                    ps = bps.tile([128, HL], F32, tag="mm", name="mmv")
                    for e in range(ECH):
                        nc.tensor.matmul(
                            ps[:],
                            xt[e][:, ts(t, 128)],
                            wv_t[:, e, :],
                            start=(e == 0),
                            stop=(e == ECH - 1),
                        )
                    nc.vector.tensor_copy(
                        out=vt[t].rearrange("p (h c) -> p h c", c=D + 1)[:, :, 0:D],
                        in_=ps.rearrange("p (h c) -> p h c", c=D),
                    )

            # ---------------- Phase C: attention, two heads in lockstep -------
            with (
                tc.tile_pool(name="cexp", bufs=4) as cexp,
                tc.tile_pool(name="cz", bufs=4) as cz,
                tc.tile_pool(name="crb", bufs=2) as crb,
                tc.tile_pool(name="csc", bufs=3, space="PSUM") as csc,
                tc.tile_pool(name="cpv", bufs=1, space="PSUM") as cpv,
            ):
                for c in range(2):  # slab = head pair (2c, 2c+1)
                    hs = [2 * c, 2 * c + 1]
                    qs = [qT[c][0:64, :], qT[c][64:128, :]]
                    ks = [kT[c][0:64, :], kT[c][64:128, :]]
                    vs = [
                        [vt[t].rearrange("p (h c) -> p h c", c=D + 1)[:, h, :]
                         for t in range(SCH)]
                        for h in hs
                    ]
                    for j in range(SB):
                        nt = 4 * (j + 1)
                        pv = [cpv.tile([65, 512], F32, tag=f"pv{i}", name=f"pv{i}")
                              for i in range(2)]
                        for tp in range(nt // 2):
                            sc = [csc.tile([128, 1024], F32, tag="sc", name="sc")
                                  for _ in range(2)]
                            # score matmuls: head 0 on PE rows 0-63, head 1 on
                            # rows 64-127 -> adjacent pairs overlap in the array
                            for half in range(2):
                                t = 2 * tp + half
                                for i in range(2):
                                    nc.tensor.matmul(
                                        sc[i][:, ts(half, 512)],
                                        ks[i][:, ts(t, 128)],
                                        qs[i][:, ts(j, 512)],
                                        start=True,
                                        stop=True,
                                    )
                            exm = []
                            for i in range(2):
                                ex = cexp.tile([128, 1024], F32R, tag="ex", name="ex")
                                nc.scalar.activation(
                                    out=ex[:], in_=sc[i][:], func=Exp, scale=0.125
                                )
                                exm.append(ex)
                            for half in range(2):
                                t = 2 * tp + half
                                if t >= nt - 4:  # diagonal chunk: causal mask
                                    for i in range(2):
                                        nc.gpsimd.affine_select(
                                            out=exm[i][:, ts(half, 512)],
                                            in_=exm[i][:, ts(half, 512)],
                                            compare_op=is_ge,
                                            fill=0.0,
                                            base=(j * 512 - t * 128),
                                            channel_multiplier=-1,
                                            pattern=[[1, 512]],
                                        )
                                for i in range(2):
                                    nc.tensor.matmul(
                                        pv[i][:],
                                        vs[i][t],
                                        exm[i][:, ts(half, 512)],
                                        start=(t == 0),
                                        stop=(t == nt - 1),
                                    )
                        zq = cz.tile([65, 2, 512], F32, tag="zq", name="zq")
                        for i in range(2):
                            nc.vector.tensor_copy(
                                out=oT[c][i * 64:(i + 1) * 64, ts(j, 512)],
                                in_=pv[i][0:64, :],
                            )
                            nc.vector.tensor_copy(
                                out=zq[64:65, i, :], in_=pv[i][64:65, :]
                            )
                        # per-block normalization, pipelined with ongoing attention:
                        # Z -> DRAM -> partition-broadcast back, invert on 128
                        # lanes, scale this block of oT in place
                        for i in range(2):
                            nc.sync.dma_start(
                                out=zscr_d[hs[i], ts(j, 512)],
                                in_=zq[64:65, i, :],
                            )
                        rb = crb.tile([128, 512], F32, tag="rb", name="rb")
                        for i in range(2):
                            nc.sync.dma_start(
                                out=rb[i * 64:(i + 1) * 64, :],
                                in_=zscr_d[hs[i]:hs[i] + 1, ts(j, 512)]
                                .to_broadcast((64, 512)),
                            )
                        rbr = crb.tile([128, 512], F32R, tag="rbr", name="rbr")
                        nc.vector.reciprocal(out=rbr[:].bitcast(F32), in_=rb[:])
                        nc.vector.tensor_mul(
                            out=oT[c][:, ts(j, 512)],
                            in0=oT[c][:, ts(j, 512)],
                            in1=rbr[:],
                        )

            # ---------------- Phase D: output projection (row-split Wo) --------
            with tc.tile_pool(name="dps", bufs=4, space="PSUM") as dps:
                for t in range(SCH):
                    for n in range(2):
                        ps = dps.tile([128, 512], F32, tag="wo", name="wops")
                        for c in range(2):
                            nc.tensor.matmul(
                                ps[:],
                                oT[c][:, ts(t, 128)],
                                wo_t[:, c, ts(n, 512)],
                                start=(c == 0),
                                stop=(c == 1),
                            )
                        ot = ev.tile([128, 512], F32, tag="out", name="oev")
                        if (2 * t + n) % 2 == 0:
                            nc.vector.tensor_copy(out=ot[:], in_=ps[:])
                        else:
                            nc.scalar.copy(out=ot[:], in_=ps[:])
                        nc.sync.dma_start(
                            out=out_d[ts(t, 128), ts(n, 512)], in_=ot[:]
                        )

    nc.compile()
    return nc


def _rope_tables():
    iexp = np.arange(0, D, 2, dtype=np.float32) / np.float32(D)
    inv_freq = np.reciprocal(np.power(np.float32(ROPE_BASE), iexp))  # (32,) f32
    ang = np.arange(S, dtype=np.float32)[:, None] * inv_freq[None, :]  # (S, 32)
    cos = np.cos(ang).astype(np.float32)  # (S, 32)
    sin = np.sin(ang).astype(np.float32)
    cosx = np.empty((64, S), dtype=np.float32)
    sinx = np.empty((64, S), dtype=np.float32)
    cosx[0::2] = cos.T
    cosx[1::2] = cos.T
    sinx[0::2] = -sin.T
    sinx[1::2] = sin.T
    return np.tile(cosx, (2, 1)), np.tile(sinx, (2, 1))  # (128, S) each


def get_nc():
    global _built
    if _built is None:
        _built = _build_nc()
    return _built


def make_in_maps(x, Wq, Wk, Wv, Wo):
    cosx, sinx = _rope_tables()
    in_maps = []
    for c in range(NCORES):
        b, g = c // 4, c % 4
        sl = slice(g * HL, (g + 1) * HL)
        in_maps.append({
            "xT": np.ascontiguousarray(x[b].T),
            "wq": np.ascontiguousarray(Wq[:, sl]),
            "wk": np.ascontiguousarray(Wk[:, sl]),
            "wv": np.ascontiguousarray(Wv[:, sl]),
            "wo": np.ascontiguousarray(Wo[sl, :]),
            "cosx": cosx,
            "sinx": sinx,
        })
    return in_maps


def gather(results):
    out = np.empty((B, S, E), dtype=np.float32)
    for b in range(B):
        acc = results[4 * b]["out"].astype(np.float32).copy()
        for g in range(1, 4):
            acc += results[4 * b + g]["out"]
        out[b] = acc
    return out


def kernel(x, Wq, Wk, Wv, Wo):
    from concourse.bass_utils import run_bass_kernel_spmd

    nc = get_nc()
    in_maps = make_in_maps(
        np.asarray(x), np.asarray(Wq), np.asarray(Wk), np.asarray(Wv), np.asarray(Wo)
    )
    res = run_bass_kernel_spmd(nc, in_maps, list(range(NCORES)))
    return gather(res.results)


# revision 11
# speedup vs baseline: 1.1926x; 1.1926x over previous
"""Multi-head causal attention with RoPE on 8 Trainium2 NeuronCores.

Sharding: tensor-parallel over heads x data-parallel over batch.
Core c handles batch b = c//4 and heads [4*(c%4), 4*(c%4)+4) (Hl=256 of Hd=1024).
Each core computes q/k/v projections for its head slice (column-split Wq/Wk/Wv),
RoPE, causal softmax attention, and a partial output projection (row-split Wo).
The host sums the 4 partial outputs per batch (the "all-reduce").

Device layouts (per core, S=2048, E=1024, Hl=256, D=64):
  xT   [E, S]    x transposed (host-side) so E rides the partition dim
  qT/kT slabs [128, S] x2: partitions = 2 heads x 64 dims, free = seq
  v    16 tiles [128, 260]: partitions = seq chunk, free = 4 heads x (64 dims + ones col)
  scores computed transposed (keys on partitions), softmax Z via ones-column of v,
  normalization by 1/Z broadcast via a DRAM-roundtrip partition-broadcast DMA.

All matmuls run in float32r (single-pass PE, ~1.5e-4 rel err measured on HW).
Attention processes the two heads of a slab in lockstep: their K=64 score
matmuls land on disjoint PE row groups (base partitions 0/64) and overlap,
and the interleave keeps the PE fed while ACT computes exp.
"""
import sys

sys.path.insert(0, "/opt/trn_rl_repo")
import numpy as np  # noqa: E402

N_HEADS = 16
B, S, E, HD = 2, 2048, 1024, 1024
D = HD // N_HEADS  # 64
HPC = 4            # heads per core
HL = HPC * D       # 256
NCORES = 8
ROPE_BASE = 10000.0

_built = None


def _build_nc():
    import concourse.bass as bass
    import concourse.tile as tile
    from concourse import bacc, mybir

    F32 = mybir.dt.float32
    F32R = mybir.dt.float32r
    F16 = mybir.dt.float16
    Exp = mybir.ActivationFunctionType.Exp
    is_ge = mybir.AluOpType.is_ge
    ts = bass.ts

    nc = bacc.Bacc("TRN2", target_bir_lowering=False, debug=False)
    xT_d = nc.dram_tensor("xT", [E, S], F32, kind="ExternalInput").ap()
    wq_d = nc.dram_tensor("wq", [E, HL], F32, kind="ExternalInput").ap()
    wk_d = nc.dram_tensor("wk", [E, HL], F32, kind="ExternalInput").ap()
    wv_d = nc.dram_tensor("wv", [E, HL], F32, kind="ExternalInput").ap()
    wo_d = nc.dram_tensor("wo", [HL, E], F16, kind="ExternalInput").ap()
    cos_d = nc.dram_tensor("cosx", [128, S], F32, kind="ExternalInput").ap()
    sin_d = nc.dram_tensor("sinx", [128, S], F32, kind="ExternalInput").ap()
    out_d = nc.dram_tensor("out", [S, E], F32, kind="ExternalOutput").ap()
    zscr_d = nc.dram_tensor("zscr", [HPC, S], F32).ap()  # internal scratch

    ECH = E // 128   # 8 e-chunks
    SCH = S // 128   # 16 seq chunks
    SB = S // 512    # 4 seq blocks
    swap_mask = []
    for i in range(16):
        swap_mask += [2 * i + 1, 2 * i]

    with tile.TileContext(nc) as tc:
        with (
            tc.tile_pool(name="persist", bufs=1) as pp,
            tc.tile_pool(name="evict", bufs=3) as ev,
        ):
            # persistent tiles
            qT = [pp.tile([128, S], F16, tag=f"qT{c}", name=f"qT{c}") for c in range(2)]
            kT = [pp.tile([128, S], F16, tag=f"kT{c}", name=f"kT{c}") for c in range(2)]
            vt = [pp.tile([128, HPC * (D + 1)], F16, tag=f"v{t}", name=f"v{t}")
                  for t in range(SCH)]
            oT = [pp.tile([128, S], F16, tag=f"oT{c}", name=f"oT{c}") for c in range(2)]
            cosx = pp.tile([128, S], F32R, tag="cosx", name="cosx")
            sinx = pp.tile([128, S], F32R, tag="sinx", name="sinx")
            wo_t = pp.tile([128, 2, E], F16, tag="wo", name="wo")

            # small/constant loads on the scalar queue, weights spread across
            # queues so nothing waits behind the big xT stream


            # ---------------- Phase B: projections + RoPE ----------------
            with (
                tc.tile_pool(name="bx", bufs=1) as bx,
                tc.tile_pool(name="bswp", bufs=2) as bswp,
                tc.tile_pool(name="bps", bufs=8, space="PSUM") as bps,
            ):
                wq_t = bx.tile([128, ECH, HL], F32R, tag="wq", name="wq")
                wk_t = bx.tile([128, ECH, HL], F32R, tag="wk", name="wk")
                wv_t = bx.tile([128, ECH, HL], F32R, tag="wv", name="wv")
                def wdma(eng, w_t_, w_d_):
                    eng.dma_start(
                        out=w_t_[:],
                        in_=w_d_.rearrange("(c p) m -> p c m", p=128).bitcast(F32R),
                    )
                xt = [bx.tile([128, S], F32R, tag=f"x{e}", name=f"x{e}")
                      for e in range(ECH)]

                def xdma(eng, e):
                    eng.dma_start(
                        out=xt[e][:], in_=xT_d[e * 128:(e + 1) * 128, :].bitcast(F32R)
                    )
                wdma(nc.sync, wq_t, wq_d)
                for e in range(0, ECH, 2):
                    xdma(nc.sync, e)
                wdma(nc.scalar, wk_t, wk_d)
                nc.scalar.dma_start(out=cosx[:], in_=cos_d.bitcast(F32R))
                nc.scalar.dma_start(out=sinx[:], in_=sin_d.bitcast(F32R))
                for e in range(1, ECH, 2):
                    xdma(nc.scalar, e)
                wdma(nc.scalar, wv_t, wv_d)
                nc.scalar.dma_start(
                    out=wo_t[:],
                    in_=wo_d.rearrange("(c p) e -> p c e", p=128),
                )

                # q/k projections -> transposed slabs, RoPE interleaved so the
                # DVE rope work overlaps the next chunk's PE matmuls
                def rope(dest, c):
                    sw = bswp.tile([128, S], F16, tag="swp", name="swp")
                    nc.vector.stream_shuffle(
                        out=sw[:], in_=dest[c][:], mask=swap_mask
                    )
                    nc.vector.tensor_mul(out=sw[:], in0=sw[:], in1=sinx[:])
                    nc.vector.tensor_mul(out=dest[c][:], in0=dest[c][:], in1=cosx[:])
                    nc.vector.tensor_add(out=dest[c][:], in0=dest[c][:], in1=sw[:])

                for w_t_, dest in ((wq_t, qT), (wk_t, kT)):
                    for m in range(2):
                        for j in range(SB):
                            ps = bps.tile([128, 512], F32, tag="mm", name="mm")
                            for e in range(ECH):
                                nc.tensor.matmul(
                                    ps[:],
                                    w_t_[:, e, m * 128:(m + 1) * 128],
                                    xt[e][:, ts(j, 512)],
                                    start=(e == 0),
                                    stop=(e == ECH - 1),
                                )
                            nc.vector.tensor_copy(
                                out=dest[m][:, ts(j, 512)], in_=ps[:]
                            )
                        rope(dest, m)
                # v projection -> seq-partition tiles with ones column
                for t in range(SCH):
                    nc.gpsimd.memset(
                        vt[t].rearrange("p (h c) -> p h c", c=D + 1)[:, :, D:D + 1],
                        1.0,
                    )
                    ps = bps.tile([128, HL], F32, tag="mm", name="mmv")
                    for e in range(ECH):
                        nc.tensor.matmul(
                            ps[:],
                            xt[e][:, ts(t, 128)],
                            wv_t[:, e, :],
                            start=(e == 0),
                            stop=(e == ECH - 1),
                        )
                    nc.vector.tensor_copy(
                        out=vt[t].rearrange("p (h c) -> p h c", c=D + 1)[:, :, 0:D],
                        in_=ps.rearrange("p (h c) -> p h c", c=D),
                    )

            # ---------------- Phase C: attention, two heads in lockstep -------
            with (
                tc.tile_pool(name="cexp", bufs=4) as cexp,
                tc.tile_pool(name="cz", bufs=4) as cz,
                tc.tile_pool(name="crb", bufs=2) as crb,
                tc.tile_pool(name="csc", bufs=3, space="PSUM") as csc,
                tc.tile_pool(name="cpv", bufs=1, space="PSUM") as cpv,
            ):
                for c in range(2):  # slab = head pair (2c, 2c+1)
                    hs = [2 * c, 2 * c + 1]
                    qs = [qT[c][0:64, :], qT[c][64:128, :]]
                    ks = [kT[c][0:64, :], kT[c][64:128, :]]
                    vs = [
                        [vt[t].rearrange("p (h c) -> p h c", c=D + 1)[:, h, :]
                         for t in range(SCH)]
                        for h in hs
                    ]
                    for j in range(SB):
                        nt = 4 * (j + 1)
                        pv = [cpv.tile([65, 512], F32, tag=f"pv{i}", name=f"pv{i}")
                              for i in range(2)]
                        for tp in range(nt // 2):
                            sc = [csc.tile([128, 1024], F32, tag="sc", name="sc")
                                  for _ in range(2)]
                            # score matmuls: head 0 on PE rows 0-63, head 1 on
                            # rows 64-127 -> adjacent pairs overlap in the array
                            for half in range(2):
                                t = 2 * tp + half
                                for i in range(2):
                                    nc.tensor.matmul(
                                        sc[i][:, ts(half, 512)],
                                        ks[i][:, ts(t, 128)],
                                        qs[i][:, ts(j, 512)],
                                        start=True,
                                        stop=True,
                                    )
                            exm = []
                            for i in range(2):
                                ex = cexp.tile([128, 1024], F16, tag="ex", name="ex")
                                nc.scalar.activation(
                                    out=ex[:], in_=sc[i][:], func=Exp, scale=0.125
                                )
                                exm.append(ex)
                            for half in range(2):
                                t = 2 * tp + half
                                if t >= nt - 4:  # diagonal chunk: causal mask
                                    for i in range(2):
                                        nc.gpsimd.affine_select(
                                            out=exm[i][:, ts(half, 512)],
                                            in_=exm[i][:, ts(half, 512)],
                                            compare_op=is_ge,
                                            fill=0.0,
                                            base=(j * 512 - t * 128),
                                            channel_multiplier=-1,
                                            pattern=[[1, 512]],
                                        )
                                for i in range(2):
                                    nc.tensor.matmul(
                                        pv[i][:],
                                        vs[i][t],
                                        exm[i][:, ts(half, 512)],
                                        start=(t == 0),
                                        stop=(t == nt - 1),
                                    )
                        zq = cz.tile([65, 2, 512], F32, tag="zq", name="zq")
                        for i in range(2):
                            nc.vector.tensor_copy(
                                out=oT[c][i * 64:(i + 1) * 64, ts(j, 512)],
                                in_=pv[i][0:64, :],
                            )
                            nc.vector.tensor_copy(
                                out=zq[64:65, i, :], in_=pv[i][64:65, :]
                            )
                        # per-block normalization, pipelined with ongoing attention:
                        # Z -> DRAM -> partition-broadcast back, invert on 128
                        # lanes, scale this block of oT in place
                        for i in range(2):
                            nc.sync.dma_start(
                                out=zscr_d[hs[i], ts(j, 512)],
                                in_=zq[64:65, i, :],
                            )
                        rb = crb.tile([128, 512], F32, tag="rb", name="rb")
                        for i in range(2):
                            nc.sync.dma_start(
                                out=rb[i * 64:(i + 1) * 64, :],
                                in_=zscr_d[hs[i]:hs[i] + 1, ts(j, 512)]
                                .to_broadcast((64, 512)),
                            )
                        rbr = crb.tile([128, 512], F32, tag="rbr", name="rbr")
                        nc.vector.reciprocal(out=rbr[:], in_=rb[:])
                        nc.vector.tensor_mul(
                            out=oT[c][:, ts(j, 512)],
                            in0=oT[c][:, ts(j, 512)],
                            in1=rbr[:],
                        )

            # ---------------- Phase D: output projection (row-split Wo) --------
            with tc.tile_pool(name="dps", bufs=4, space="PSUM") as dps:
                for t in range(SCH):
                    for n in range(2):
                        ps = dps.tile([128, 512], F32, tag="wo", name="wops")
                        for c in range(2):
                            nc.tensor.matmul(
                                ps[:],
                                oT[c][:, ts(t, 128)],
                                wo_t[:, c, ts(n, 512)],
                                start=(c == 0),
                                stop=(c == 1),
                            )
                        ot = ev.tile([128, 512], F32, tag="out", name="oev")
                        if (2 * t + n) % 2 == 0:
                            nc.vector.tensor_copy(out=ot[:], in_=ps[:])
                        else:
                            nc.scalar.copy(out=ot[:], in_=ps[:])
                        nc.sync.dma_start(
                            out=out_d[ts(t, 128), ts(n, 512)], in_=ot[:]
                        )

    nc.compile()
    return nc


def _rope_tables():
    iexp = np.arange(0, D, 2, dtype=np.float32) / np.float32(D)
    inv_freq = np.reciprocal(np.power(np.float32(ROPE_BASE), iexp))  # (32,) f32
    ang = np.arange(S, dtype=np.float32)[:, None] * inv_freq[None, :]  # (S, 32)
    cos = np.cos(ang).astype(np.float32)  # (S, 32)
    sin = np.sin(ang).astype(np.float32)
    cosx = np.empty((64, S), dtype=np.float32)
    sinx = np.empty((64, S), dtype=np.float32)
    cosx[0::2] = cos.T
    cosx[1::2] = cos.T
    sinx[0::2] = -sin.T
    sinx[1::2] = sin.T
    return np.tile(cosx, (2, 1)), np.tile(sinx, (2, 1))  # (128, S) each


def get_nc():
    global _built
    if _built is None:
        _built = _build_nc()
    return _built


def make_in_maps(x, Wq, Wk, Wv, Wo):
    cosx, sinx = _rope_tables()
    in_maps = []
    for c in range(NCORES):
        b, g = c // 4, c % 4
        sl = slice(g * HL, (g + 1) * HL)
        in_maps.append({
            "xT": np.ascontiguousarray(x[b].T),
            "wq": np.ascontiguousarray(Wq[:, sl]),
            "wk": np.ascontiguousarray(Wk[:, sl]),
            "wv": np.ascontiguousarray(Wv[:, sl]),
            "wo": np.ascontiguousarray(Wo[sl, :]).astype(np.float16),
            "cosx": cosx,
            "sinx": sinx,
        })
    return in_maps


def gather(results):
    out = np.empty((B, S, E), dtype=np.float32)
    for b in range(B):
        acc = results[4 * b]["out"].astype(np.float32).copy()
        for g in range(1, 4):
            acc += results[4 * b + g]["out"]
        out[b] = acc
    return out


def kernel(x, Wq, Wk, Wv, Wo):
    from concourse.bass_utils import run_bass_kernel_spmd

    nc = get_nc()
    in_maps = make_in_maps(
        np.asarray(x), np.asarray(Wq), np.asarray(Wk), np.asarray(Wv), np.asarray(Wo)
    )
    res = run_bass_kernel_spmd(nc, in_maps, list(range(NCORES)))
    return gather(res.results)


# revision 12
# speedup vs baseline: 1.3742x; 1.1523x over previous
"""Multi-head causal attention with RoPE on 8 Trainium2 NeuronCores.

Sharding: tensor-parallel over heads x data-parallel over batch.
Core c handles batch b = c//4 and heads [4*(c%4), 4*(c%4)+4) (Hl=256 of Hd=1024).
Each core computes q/k/v projections for its head slice (column-split Wq/Wk/Wv),
RoPE, causal softmax attention, and a partial output projection (row-split Wo).
The host sums the 4 partial outputs per batch (the "all-reduce").

Device layouts (per core, S=2048, E=1024, Hl=256, D=64):
  xT   [E, S]    x transposed (host-side) so E rides the partition dim
  qT/kT slabs [128, S] x2: partitions = 2 heads x 64 dims, free = seq
  v    16 tiles [128, 260]: partitions = seq chunk, free = 4 heads x (64 dims + ones col)
  scores computed transposed (keys on partitions), softmax Z via ones-column of v,
  normalization by 1/Z broadcast via a DRAM-roundtrip partition-broadcast DMA.

All matmuls run in float32r (single-pass PE, ~1.5e-4 rel err measured on HW).
Attention processes the two heads of a slab in lockstep: their K=64 score
matmuls land on disjoint PE row groups (base partitions 0/64) and overlap,
and the interleave keeps the PE fed while ACT computes exp.
"""
import sys

sys.path.insert(0, "/opt/trn_rl_repo")
import numpy as np  # noqa: E402

N_HEADS = 16
B, S, E, HD = 2, 2048, 1024, 1024
D = HD // N_HEADS  # 64
HPC = 4            # heads per core
HL = HPC * D       # 256
NCORES = 8
ROPE_BASE = 10000.0

_built = None


def _build_nc():
    import concourse.bass as bass
    import concourse.tile as tile
    from concourse import bacc, mybir

    F32 = mybir.dt.float32
    F32R = mybir.dt.float32r
    F16 = mybir.dt.float16
    Exp = mybir.ActivationFunctionType.Exp
    is_ge = mybir.AluOpType.is_ge
    ts = bass.ts

    nc = bacc.Bacc("TRN2", target_bir_lowering=False, debug=False)
    xT_d = nc.dram_tensor("xT", [E, S], F16, kind="ExternalInput").ap()
    wq_d = nc.dram_tensor("wq", [E, HL], F16, kind="ExternalInput").ap()
    wk_d = nc.dram_tensor("wk", [E, HL], F16, kind="ExternalInput").ap()
    wv_d = nc.dram_tensor("wv", [E, HL], F16, kind="ExternalInput").ap()
    wo_d = nc.dram_tensor("wo", [HL, E], F16, kind="ExternalInput").ap()
    cos_d = nc.dram_tensor("cosx", [128, S], F32, kind="ExternalInput").ap()
    sin_d = nc.dram_tensor("sinx", [128, S], F32, kind="ExternalInput").ap()
    out_d = nc.dram_tensor("out", [S, E], F32, kind="ExternalOutput").ap()
    zscr_d = nc.dram_tensor("zscr", [HPC, S], F32).ap()  # internal scratch

    ECH = E // 128   # 8 e-chunks
    SCH = S // 128   # 16 seq chunks
    SB = S // 512    # 4 seq blocks
    swap_mask = []
    for i in range(16):
        swap_mask += [2 * i + 1, 2 * i]

    with tile.TileContext(nc) as tc:
        with (
            tc.tile_pool(name="persist", bufs=1) as pp,
            tc.tile_pool(name="evict", bufs=6) as ev,
        ):
            # persistent tiles
            qT = [pp.tile([128, S], F16, tag=f"qT{c}", name=f"qT{c}") for c in range(2)]
            kT = [pp.tile([128, S], F16, tag=f"kT{c}", name=f"kT{c}") for c in range(2)]
            vt = [pp.tile([128, HPC * (D + 1)], F16, tag=f"v{t}", name=f"v{t}")
                  for t in range(SCH)]
            oT = [pp.tile([128, S], F16, tag=f"oT{c}", name=f"oT{c}") for c in range(2)]
            cosx = pp.tile([128, S], F32R, tag="cosx", name="cosx")
            sinx = pp.tile([128, S], F32R, tag="sinx", name="sinx")
            wo_t = pp.tile([128, 2, E], F16, tag="wo", name="wo")

            # small/constant loads on the scalar queue, weights spread across
            # queues so nothing waits behind the big xT stream


            # ---------------- Phase B: projections + RoPE ----------------
            with (
                tc.tile_pool(name="bx", bufs=1) as bx,
                tc.tile_pool(name="bswp", bufs=2) as bswp,
                tc.tile_pool(name="bps", bufs=8, space="PSUM") as bps,
            ):
                wq_t = bx.tile([128, ECH, HL], F16, tag="wq", name="wq")
                wk_t = bx.tile([128, ECH, HL], F16, tag="wk", name="wk")
                wv_t = bx.tile([128, ECH, HL], F16, tag="wv", name="wv")
                def wdma(eng, w_t_, w_d_):
                    eng.dma_start(
                        out=w_t_[:],
                        in_=w_d_.rearrange("(c p) m -> p c m", p=128),
                    )
                xt = [bx.tile([128, S], F16, tag=f"x{e}", name=f"x{e}")
                      for e in range(ECH)]

                def xdma(eng, e):
                    eng.dma_start(
                        out=xt[e][:], in_=xT_d[e * 128:(e + 1) * 128, :]
                    )
                wdma(nc.sync, wq_t, wq_d)
                for e in range(0, ECH, 2):
                    xdma(nc.sync, e)
                wdma(nc.scalar, wk_t, wk_d)
                nc.scalar.dma_start(out=cosx[:], in_=cos_d.bitcast(F32R))
                nc.scalar.dma_start(out=sinx[:], in_=sin_d.bitcast(F32R))
                for e in range(1, ECH, 2):
                    xdma(nc.scalar, e)
                wdma(nc.scalar, wv_t, wv_d)
                nc.scalar.dma_start(
                    out=wo_t[:],
                    in_=wo_d.rearrange("(c p) e -> p c e", p=128),
                )

                # q/k projections -> transposed slabs, RoPE interleaved so the
                # DVE rope work overlaps the next chunk's PE matmuls
                def rope(dest, c):
                    sw = bswp.tile([128, S], F16, tag="swp", name="swp")
                    nc.vector.stream_shuffle(
                        out=sw[:], in_=dest[c][:], mask=swap_mask
                    )
                    nc.vector.tensor_mul(out=sw[:], in0=sw[:], in1=sinx[:])
                    nc.vector.tensor_mul(out=dest[c][:], in0=dest[c][:], in1=cosx[:])
                    nc.vector.tensor_add(out=dest[c][:], in0=dest[c][:], in1=sw[:])

                for w_t_, dest in ((wq_t, qT), (wk_t, kT)):
                    for m in range(2):
                        for j in range(SB):
                            ps = bps.tile([128, 512], F32, tag="mm", name="mm")
                            for e in range(ECH):
                                nc.tensor.matmul(
                                    ps[:],
                                    w_t_[:, e, m * 128:(m + 1) * 128],
                                    xt[e][:, ts(j, 512)],
                                    start=(e == 0),
                                    stop=(e == ECH - 1),
                                )
                            nc.vector.tensor_copy(
                                out=dest[m][:, ts(j, 512)], in_=ps[:]
                            )
                        rope(dest, m)
                # v projection -> seq-partition tiles with ones column
                for t in range(SCH):
                    nc.gpsimd.memset(
                        vt[t].rearrange("p (h c) -> p h c", c=D + 1)[:, :, D:D + 1],
                        1.0,
                    )
                    ps = bps.tile([128, HL], F32, tag="mm", name="mmv")
                    for e in range(ECH):
                        nc.tensor.matmul(
                            ps[:],
                            xt[e][:, ts(t, 128)],
                            wv_t[:, e, :],
                            start=(e == 0),
                            stop=(e == ECH - 1),
                        )
                    nc.vector.tensor_copy(
                        out=vt[t].rearrange("p (h c) -> p h c", c=D + 1)[:, :, 0:D],
                        in_=ps.rearrange("p (h c) -> p h c", c=D),
                    )

            # ---------------- Phase C: attention, two heads in lockstep -------
            with (
                tc.tile_pool(name="cexp", bufs=4) as cexp,
                tc.tile_pool(name="cz", bufs=4) as cz,
                tc.tile_pool(name="crb", bufs=2) as crb,
                tc.tile_pool(name="csc", bufs=3, space="PSUM") as csc,
                tc.tile_pool(name="cpv", bufs=1, space="PSUM") as cpv,
            ):
                for c in range(2):  # slab = head pair (2c, 2c+1)
                    hs = [2 * c, 2 * c + 1]
                    qs = [qT[c][0:64, :], qT[c][64:128, :]]
                    ks = [kT[c][0:64, :], kT[c][64:128, :]]
                    vs = [
                        [vt[t].rearrange("p (h c) -> p h c", c=D + 1)[:, h, :]
                         for t in range(SCH)]
                        for h in hs
                    ]
                    for j in range(SB):
                        nt = 4 * (j + 1)
                        pv = [cpv.tile([65, 512], F32, tag=f"pv{i}", name=f"pv{i}")
                              for i in range(2)]
                        for tp in range(nt // 2):
                            sc = [csc.tile([128, 1024], F32, tag="sc", name="sc")
                                  for _ in range(2)]
                            # score matmuls: head 0 on PE rows 0-63, head 1 on
                            # rows 64-127 -> adjacent pairs overlap in the array
                            for half in range(2):
                                t = 2 * tp + half
                                for i in range(2):
                                    nc.tensor.matmul(
                                        sc[i][:, ts(half, 512)],
                                        ks[i][:, ts(t, 128)],
                                        qs[i][:, ts(j, 512)],
                                        start=True,
                                        stop=True,
                                    )
                            exm = []
                            for i in range(2):
                                ex = cexp.tile([128, 1024], F16, tag="ex", name="ex")
                                nc.scalar.activation(
                                    out=ex[:], in_=sc[i][:], func=Exp, scale=0.125
                                )
                                exm.append(ex)
                            for half in range(2):
                                t = 2 * tp + half
                                if t >= nt - 4:  # diagonal chunk: causal mask
                                    for i in range(2):
                                        nc.gpsimd.affine_select(
                                            out=exm[i][:, ts(half, 512)],
                                            in_=exm[i][:, ts(half, 512)],
                                            compare_op=is_ge,
                                            fill=0.0,
                                            base=(j * 512 - t * 128),
                                            channel_multiplier=-1,
                                            pattern=[[1, 512]],
                                        )
                                for i in range(2):
                                    nc.tensor.matmul(
                                        pv[i][:],
                                        vs[i][t],
                                        exm[i][:, ts(half, 512)],
                                        start=(t == 0),
                                        stop=(t == nt - 1),
                                    )
                        zq = cz.tile([65, 2, 512], F32, tag="zq", name="zq")
                        for i in range(2):
                            nc.vector.tensor_copy(
                                out=oT[c][i * 64:(i + 1) * 64, ts(j, 512)],
                                in_=pv[i][0:64, :],
                            )
                            nc.vector.tensor_copy(
                                out=zq[64:65, i, :], in_=pv[i][64:65, :]
                            )
                        # per-block normalization, pipelined with ongoing attention:
                        # Z -> DRAM -> partition-broadcast back, invert on 128
                        # lanes, scale this block of oT in place
                        for i in range(2):
                            nc.sync.dma_start(
                                out=zscr_d[hs[i], ts(j, 512)],
                                in_=zq[64:65, i, :],
                            )
                        rb = crb.tile([128, 512], F32, tag="rb", name="rb")
                        for i in range(2):
                            nc.sync.dma_start(
                                out=rb[i * 64:(i + 1) * 64, :],
                                in_=zscr_d[hs[i]:hs[i] + 1, ts(j, 512)]
                                .to_broadcast((64, 512)),
                            )
                        rbr = crb.tile([128, 512], F32, tag="rbr", name="rbr")
                        nc.vector.reciprocal(out=rbr[:], in_=rb[:])
                        nc.vector.tensor_mul(
                            out=oT[c][:, ts(j, 512)],
                            in0=oT[c][:, ts(j, 512)],
                            in1=rbr[:],
                        )

            # ---------------- Phase D: output projection (row-split Wo) --------
            with tc.tile_pool(name="dps", bufs=8, space="PSUM") as dps:
                for t in range(SCH):
                    for n in range(2):
                        ps = dps.tile([128, 512], F32, tag="wo", name="wops")
                        for c in range(2):
                            nc.tensor.matmul(
                                ps[:],
                                oT[c][:, ts(t, 128)],
                                wo_t[:, c, ts(n, 512)],
                                start=(c == 0),
                                stop=(c == 1),
                            )
                        ot = ev.tile([128, 512], F32, tag="out", name="oev")
                        if (2 * t + n) % 2 == 0:
                            nc.vector.tensor_copy(out=ot[:], in_=ps[:])
                        else:
                            nc.scalar.copy(out=ot[:], in_=ps[:])
                        nc.sync.dma_start(
                            out=out_d[ts(t, 128), ts(n, 512)], in_=ot[:]
                        )

    nc.compile()
    return nc


def _rope_tables():
    iexp = np.arange(0, D, 2, dtype=np.float32) / np.float32(D)
    inv_freq = np.reciprocal(np.power(np.float32(ROPE_BASE), iexp))  # (32,) f32
    ang = np.arange(S, dtype=np.float32)[:, None] * inv_freq[None, :]  # (S, 32)
    cos = np.cos(ang).astype(np.float32)  # (S, 32)
    sin = np.sin(ang).astype(np.float32)
    cosx = np.empty((64, S), dtype=np.float32)
    sinx = np.empty((64, S), dtype=np.float32)
    cosx[0::2] = cos.T
    cosx[1::2] = cos.T
    sinx[0::2] = -sin.T
    sinx[1::2] = sin.T
    return np.tile(cosx, (2, 1)), np.tile(sinx, (2, 1))  # (128, S) each


def get_nc():
    global _built
    if _built is None:
        _built = _build_nc()
    return _built


def make_in_maps(x, Wq, Wk, Wv, Wo):
    cosx, sinx = _rope_tables()
    in_maps = []
    for c in range(NCORES):
        b, g = c // 4, c % 4
        sl = slice(g * HL, (g + 1) * HL)
        in_maps.append({
            "xT": np.ascontiguousarray(x[b].T).astype(np.float16),
            "wq": np.ascontiguousarray(Wq[:, sl]).astype(np.float16),
            "wk": np.ascontiguousarray(Wk[:, sl]).astype(np.float16),
            "wv": np.ascontiguousarray(Wv[:, sl]).astype(np.float16),
            "wo": np.ascontiguousarray(Wo[sl, :]).astype(np.float16),
            "cosx": cosx,
            "sinx": sinx,
        })
    return in_maps


def gather(results):
    out = np.empty((B, S, E), dtype=np.float32)
    for b in range(B):
        acc = results[4 * b]["out"].astype(np.float32).copy()
        for g in range(1, 4):
            acc += results[4 * b + g]["out"]
        out[b] = acc
    return out


def kernel(x, Wq, Wk, Wv, Wo):
    from concourse.bass_utils import run_bass_kernel_spmd

    nc = get_nc()
    in_maps = make_in_maps(
        np.asarray(x), np.asarray(Wq), np.asarray(Wk), np.asarray(Wv), np.asarray(Wo)
    )
    res = run_bass_kernel_spmd(nc, in_maps, list(range(NCORES)))
    return gather(res.results)


# revision 13
# speedup vs baseline: 1.3801x; 1.0043x over previous
"""Multi-head causal attention with RoPE on 8 Trainium2 NeuronCores.

Sharding: tensor-parallel over heads x data-parallel over batch.
Core c handles batch b = c//4 and heads [4*(c%4), 4*(c%4)+4) (Hl=256 of Hd=1024).
Each core computes q/k/v projections for its head slice (column-split Wq/Wk/Wv),
RoPE, causal softmax attention, and a partial output projection (row-split Wo).
The host sums the 4 partial outputs per batch (the "all-reduce").

Device layouts (per core, S=2048, E=1024, Hl=256, D=64):
  xT   [E, S]    x transposed (host-side) so E rides the partition dim
  qT/kT slabs [128, S] x2: partitions = 2 heads x 64 dims, free = seq
  v    16 tiles [128, 260]: partitions = seq chunk, free = 4 heads x (64 dims + ones col)
  scores computed transposed (keys on partitions), softmax Z via ones-column of v,
  normalization by 1/Z broadcast via a DRAM-roundtrip partition-broadcast DMA.

All matmuls run in float32r (single-pass PE, ~1.5e-4 rel err measured on HW).
Attention processes the two heads of a slab in lockstep: their K=64 score
matmuls land on disjoint PE row groups (base partitions 0/64) and overlap,
and the interleave keeps the PE fed while ACT computes exp.
"""
import sys

sys.path.insert(0, "/opt/trn_rl_repo")
import numpy as np  # noqa: E402

N_HEADS = 16
B, S, E, HD = 2, 2048, 1024, 1024
D = HD // N_HEADS  # 64
HPC = 4            # heads per core
HL = HPC * D       # 256
NCORES = 8
ROPE_BASE = 10000.0

_built = None


def _build_nc():
    import concourse.bass as bass
    import concourse.tile as tile
    from concourse import bacc, mybir

    F32 = mybir.dt.float32
    F32R = mybir.dt.float32r
    F16 = mybir.dt.float16
    Exp = mybir.ActivationFunctionType.Exp
    is_ge = mybir.AluOpType.is_ge
    ts = bass.ts

    nc = bacc.Bacc("TRN2", target_bir_lowering=False, debug=False)
    xT_d = nc.dram_tensor("xT", [E, S], F16, kind="ExternalInput").ap()
    wq_d = nc.dram_tensor("wq", [E, HL], F16, kind="ExternalInput").ap()
    wk_d = nc.dram_tensor("wk", [E, HL], F16, kind="ExternalInput").ap()
    wv_d = nc.dram_tensor("wv", [E, HL], F16, kind="ExternalInput").ap()
    wo_d = nc.dram_tensor("wo", [HL, E], F16, kind="ExternalInput").ap()
    cos_d = nc.dram_tensor("cosx", [128, S], F32, kind="ExternalInput").ap()
    sin_d = nc.dram_tensor("sinx", [128, S], F32, kind="ExternalInput").ap()
    out_d = nc.dram_tensor("out", [S, E], F32, kind="ExternalOutput").ap()
    zscr_d = nc.dram_tensor("zscr", [HPC, S], F32).ap()  # internal scratch

    ECH = E // 128   # 8 e-chunks
    SCH = S // 128   # 16 seq chunks
    SB = S // 512    # 4 seq blocks
    swap_mask = []
    for i in range(16):
        swap_mask += [2 * i + 1, 2 * i]

    with tile.TileContext(nc) as tc:
        with (
            tc.tile_pool(name="persist", bufs=1) as pp,
            tc.tile_pool(name="evict", bufs=6) as ev,
        ):
            # persistent tiles
            qT = [pp.tile([128, S], F16, tag=f"qT{c}", name=f"qT{c}") for c in range(2)]
            kT = [pp.tile([128, S], F16, tag=f"kT{c}", name=f"kT{c}") for c in range(2)]
            vt = [pp.tile([128, HPC * (D + 1)], F16, tag=f"v{t}", name=f"v{t}")
                  for t in range(SCH)]
            oT = [pp.tile([128, S], F16, tag=f"oT{c}", name=f"oT{c}") for c in range(2)]
            cosx = pp.tile([128, S], F32R, tag="cosx", name="cosx")
            sinx = pp.tile([128, S], F32R, tag="sinx", name="sinx")
            wo_t = pp.tile([128, 2, E], F16, tag="wo", name="wo")

            # small/constant loads on the scalar queue, weights spread across
            # queues so nothing waits behind the big xT stream


            # ---------------- Phase B: projections + RoPE ----------------
            with (
                tc.tile_pool(name="bx", bufs=1) as bx,
                tc.tile_pool(name="bswp", bufs=2) as bswp,
                tc.tile_pool(name="bps", bufs=8, space="PSUM") as bps,
            ):
                wq_t = bx.tile([128, ECH, HL], F16, tag="wq", name="wq")
                wk_t = bx.tile([128, ECH, HL], F16, tag="wk", name="wk")
                wv_t = bx.tile([128, ECH, HL], F16, tag="wv", name="wv")
                def wdma(eng, w_t_, w_d_):
                    eng.dma_start(
                        out=w_t_[:],
                        in_=w_d_.rearrange("(c p) m -> p c m", p=128),
                    )
                xt = [bx.tile([128, S], F16, tag=f"x{e}", name=f"x{e}")
                      for e in range(ECH)]

                def xdma(eng, e, j):
                    eng.dma_start(
                        out=xt[e][:, ts(j, 512)],
                        in_=xT_d[e * 128:(e + 1) * 128, ts(j, 512)],
                    )
                # feed order: j-block 0 slices of every chunk first, so the
                # first projection psum group completes within a few us
                wdma(nc.sync, wq_t, wq_d)
                wdma(nc.scalar, wk_t, wk_d)
                for e in range(0, ECH, 2):
                    xdma(nc.sync, e, 0)
                for e in range(1, ECH, 2):
                    xdma(nc.scalar, e, 0)
                for e in range(0, ECH, 2):
                    xdma(nc.sync, e, 1)
                nc.scalar.dma_start(out=cosx[:], in_=cos_d.bitcast(F32R))
                nc.scalar.dma_start(out=sinx[:], in_=sin_d.bitcast(F32R))
                for e in range(1, ECH, 2):
                    xdma(nc.scalar, e, 1)
                wdma(nc.scalar, wv_t, wv_d)
                for j in range(2, SB):
                    for e in range(0, ECH, 2):
                        xdma(nc.sync, e, j)
                    for e in range(1, ECH, 2):
                        xdma(nc.scalar, e, j)
                nc.scalar.dma_start(
                    out=wo_t[:],
                    in_=wo_d.rearrange("(c p) e -> p c e", p=128),
                )

                # q/k projections -> transposed slabs, RoPE interleaved so the
                # DVE rope work overlaps the next chunk's PE matmuls
                def rope(dest, c):
                    sw = bswp.tile([128, S], F16, tag="swp", name="swp")
                    nc.vector.stream_shuffle(
                        out=sw[:], in_=dest[c][:], mask=swap_mask
                    )
                    nc.vector.tensor_mul(out=sw[:], in0=sw[:], in1=sinx[:])
                    nc.vector.tensor_mul(out=dest[c][:], in0=dest[c][:], in1=cosx[:])
                    nc.vector.tensor_add(out=dest[c][:], in0=dest[c][:], in1=sw[:])

                for w_t_, dest in ((wq_t, qT), (wk_t, kT)):
                    for m in range(2):
                        for j in range(SB):
                            ps = bps.tile([128, 512], F32, tag="mm", name="mm")
                            for e in range(ECH):
                                nc.tensor.matmul(
                                    ps[:],
                                    w_t_[:, e, m * 128:(m + 1) * 128],
                                    xt[e][:, ts(j, 512)],
                                    start=(e == 0),
                                    stop=(e == ECH - 1),
                                )
                            nc.vector.tensor_copy(
                                out=dest[m][:, ts(j, 512)], in_=ps[:]
                            )
                        if m == 0:
                            rope(dest, m)
                # v projection -> seq-partition tiles with ones column
                for t in range(SCH):
                    nc.gpsimd.memset(
                        vt[t].rearrange("p (h c) -> p h c", c=D + 1)[:, :, D:D + 1],
                        1.0,
                    )
                    ps = bps.tile([128, HL], F32, tag="mm", name="mmv")
                    for e in range(ECH):
                        nc.tensor.matmul(
                            ps[:],
                            xt[e][:, ts(t, 128)],
                            wv_t[:, e, :],
                            start=(e == 0),
                            stop=(e == ECH - 1),
                        )
                    nc.vector.tensor_copy(
                        out=vt[t].rearrange("p (h c) -> p h c", c=D + 1)[:, :, 0:D],
                        in_=ps.rearrange("p (h c) -> p h c", c=D),
                    )
                rope(qT, 1)
                rope(kT, 1)

            # ---------------- Phase C: attention, two heads in lockstep -------
            with (
                tc.tile_pool(name="cexp", bufs=4) as cexp,
                tc.tile_pool(name="cz", bufs=4) as cz,
                tc.tile_pool(name="crb", bufs=2) as crb,
                tc.tile_pool(name="csc", bufs=3, space="PSUM") as csc,
                tc.tile_pool(name="cpv", bufs=1, space="PSUM") as cpv,
            ):
                for c in range(2):  # slab = head pair (2c, 2c+1)
                    hs = [2 * c, 2 * c + 1]
                    qs = [qT[c][0:64, :], qT[c][64:128, :]]
                    ks = [kT[c][0:64, :], kT[c][64:128, :]]
                    vs = [
                        [vt[t].rearrange("p (h c) -> p h c", c=D + 1)[:, h, :]
                         for t in range(SCH)]
                        for h in hs
                    ]
                    for j in range(SB):
                        nt = 4 * (j + 1)
                        pv = [cpv.tile([65, 512], F32, tag=f"pv{i}", name=f"pv{i}")
                              for i in range(2)]
                        for tp in range(nt // 2):
                            sc = [csc.tile([128, 1024], F32, tag="sc", name="sc")
                                  for _ in range(2)]
                            # score matmuls: head 0 on PE rows 0-63, head 1 on
                            # rows 64-127 -> adjacent pairs overlap in the array
                            for half in range(2):
                                t = 2 * tp + half
                                for i in range(2):
                                    nc.tensor.matmul(
                                        sc[i][:, ts(half, 512)],
                                        ks[i][:, ts(t, 128)],
                                        qs[i][:, ts(j, 512)],
                                        start=True,
                                        stop=True,
                                    )
                            exm = []
                            for i in range(2):
                                ex = cexp.tile([128, 1024], F16, tag="ex", name="ex")
                                nc.scalar.activation(
                                    out=ex[:], in_=sc[i][:], func=Exp, scale=0.125
                                )
                                exm.append(ex)
                            for half in range(2):
                                t = 2 * tp + half
                                if t >= nt - 4:  # diagonal chunk: causal mask
                                    for i in range(2):
                                        nc.gpsimd.affine_select(
                                            out=exm[i][:, ts(half, 512)],
                                            in_=exm[i][:, ts(half, 512)],
                                            compare_op=is_ge,
                                            fill=0.0,
                                            base=(j * 512 - t * 128),
                                            channel_multiplier=-1,
                                            pattern=[[1, 512]],
                                        )
                                for i in range(2):
                                    nc.tensor.matmul(
                                        pv[i][:],
                                        vs[i][t],
                                        exm[i][:, ts(half, 512)],
                                        start=(t == 0),
                                        stop=(t == nt - 1),
                                    )
                        zq = cz.tile([65, 2, 512], F32, tag="zq", name="zq")
                        for i in range(2):
                            nc.vector.tensor_copy(
                                out=oT[c][i * 64:(i + 1) * 64, ts(j, 512)],
                                in_=pv[i][0:64, :],
                            )
                            nc.vector.tensor_copy(
                                out=zq[64:65, i, :], in_=pv[i][64:65, :]
                            )
                        # per-block normalization, pipelined with ongoing attention:
                        # Z -> DRAM -> partition-broadcast back, invert on 128
                        # lanes, scale this block of oT in place
                        for i in range(2):
                            nc.sync.dma_start(
                                out=zscr_d[hs[i], ts(j, 512)],
                                in_=zq[64:65, i, :],
                            )
                        rb = crb.tile([128, 512], F32, tag="rb", name="rb")
                        for i in range(2):
                            nc.sync.dma_start(
                                out=rb[i * 64:(i + 1) * 64, :],
                                in_=zscr_d[hs[i]:hs[i] + 1, ts(j, 512)]
                                .to_broadcast((64, 512)),
                            )
                        rbr = crb.tile([128, 512], F32, tag="rbr", name="rbr")
                        nc.vector.reciprocal(out=rbr[:], in_=rb[:])
                        nc.vector.tensor_mul(
                            out=oT[c][:, ts(j, 512)],
                            in0=oT[c][:, ts(j, 512)],
                            in1=rbr[:],
                        )

            # ---------------- Phase D: output projection (row-split Wo) --------
            with tc.tile_pool(name="dps", bufs=8, space="PSUM") as dps:
                for t in range(SCH):
                    for n in range(2):
                        ps = dps.tile([128, 512], F32, tag="wo", name="wops")
                        for c in range(2):
                            nc.tensor.matmul(
                                ps[:],
                                oT[c][:, ts(t, 128)],
                                wo_t[:, c, ts(n, 512)],
                                start=(c == 0),
                                stop=(c == 1),
                            )
                        ot = ev.tile([128, 512], F32, tag="out", name="oev")
                        if (2 * t + n) % 2 == 0:
                            nc.vector.tensor_copy(out=ot[:], in_=ps[:])
                        else:
                            nc.scalar.copy(out=ot[:], in_=ps[:])
                        nc.sync.dma_start(
                            out=out_d[ts(t, 128), ts(n, 512)], in_=ot[:]
                        )

    nc.compile()
    return nc


def _rope_tables():
    iexp = np.arange(0, D, 2, dtype=np.float32) / np.float32(D)
    inv_freq = np.reciprocal(np.power(np.float32(ROPE_BASE), iexp))  # (32,) f32
    ang = np.arange(S, dtype=np.float32)[:, None] * inv_freq[None, :]  # (S, 32)
    cos = np.cos(ang).astype(np.float32)  # (S, 32)
    sin = np.sin(ang).astype(np.float32)
    cosx = np.empty((64, S), dtype=np.float32)
    sinx = np.empty((64, S), dtype=np.float32)
    cosx[0::2] = cos.T
    cosx[1::2] = cos.T
    sinx[0::2] = -sin.T
    sinx[1::2] = sin.T
    return np.tile(cosx, (2, 1)), np.tile(sinx, (2, 1))  # (128, S) each


def get_nc():
    global _built
    if _built is None:
        _built = _build_nc()
    return _built


def make_in_maps(x, Wq, Wk, Wv, Wo):
    cosx, sinx = _rope_tables()
    in_maps = []
    for c in range(NCORES):
        b, g = c // 4, c % 4
        sl = slice(g * HL, (g + 1) * HL)
        in_maps.append({
            "xT": np.ascontiguousarray(x[b].T).astype(np.float16),
            "wq": np.ascontiguousarray(Wq[:, sl]).astype(np.float16),
            "wk": np.ascontiguousarray(Wk[:, sl]).astype(np.float16),
            "wv": np.ascontiguousarray(Wv[:, sl]).astype(np.float16),
            "wo": np.ascontiguousarray(Wo[sl, :]).astype(np.float16),
            "cosx": cosx,
            "sinx": sinx,
        })
    return in_maps


def gather(results):
    out = np.empty((B, S, E), dtype=np.float32)
    for b in range(B):
        acc = results[4 * b]["out"].astype(np.float32).copy()
        for g in range(1, 4):
            acc += results[4 * b + g]["out"]
        out[b] = acc
    return out


def kernel(x, Wq, Wk, Wv, Wo):
    from concourse.bass_utils import run_bass_kernel_spmd

    nc = get_nc()
    in_maps = make_in_maps(
        np.asarray(x), np.asarray(Wq), np.asarray(Wk), np.asarray(Wv), np.asarray(Wo)
    )
    res = run_bass_kernel_spmd(nc, in_maps, list(range(NCORES)))
    return gather(res.results)


# revision 20
# speedup vs baseline: 1.4254x; 1.0328x over previous
"""Multi-head causal attention with RoPE on 8 Trainium2 NeuronCores.

Sharding: tensor-parallel over heads x data-parallel over batch.
Core c handles batch b = c//4 and heads [4*(c%4), 4*(c%4)+4) (Hl=256 of Hd=1024).
Each core computes q/k/v projections for its head slice (column-split Wq/Wk/Wv),
RoPE, causal softmax attention, and a partial output projection (row-split Wo).
The host sums the 4 partial outputs per batch (the "all-reduce").

Device layouts (per core, S=2048, E=1024, Hl=256, D=64):
  xT   [E, S]    x transposed (host-side) so E rides the partition dim
  qT/kT slabs [128, S] x2: partitions = 2 heads x 64 dims, free = seq
  v    16 tiles [128, 260]: partitions = seq chunk, free = 4 heads x (64 dims + ones col)
  scores computed transposed (keys on partitions), softmax Z via ones-column of v,
  normalization by 1/Z broadcast via a DRAM-roundtrip partition-broadcast DMA,
  pipelined per 512-column block.

All matmul operands are fp16 (PE runs 1 cyc/row with fast weight loads; fp32
PSUM accumulation; measured end-to-end rel err ~5e-4). The two heads of a slab
run in lockstep: their K=64 score matmuls land on disjoint PE row groups (base
partitions 0/64) and overlap in the array, and the interleave keeps the PE fed
while ACT computes exp. The v projection is interleaved into slab-0 attention
and the Wo projection into slab-1 attention so independent PE work fills the
exp-bound stretches.
"""
import sys

sys.path.insert(0, "/opt/trn_rl_repo")
import numpy as np  # noqa: E402

N_HEADS = 16
B, S, E, HD = 2, 2048, 1024, 1024
D = HD // N_HEADS  # 64
HPC = 4            # heads per core
HL = HPC * D       # 256
NCORES = 8
ROPE_BASE = 10000.0

_built = None


def _build_nc():
    import concourse.bass as bass
    import concourse.tile as tile
    from concourse import bacc, mybir

    F32 = mybir.dt.float32
    F32R = mybir.dt.float32r
    F16 = mybir.dt.float16
    Exp = mybir.ActivationFunctionType.Exp
    is_ge = mybir.AluOpType.is_ge
    ts = bass.ts

    nc = bacc.Bacc("TRN2", target_bir_lowering=False, debug=False)
    xT_d = nc.dram_tensor("xT", [E, S], F16, kind="ExternalInput").ap()
    wq_d = nc.dram_tensor("wq", [E, HL], F16, kind="ExternalInput").ap()
    wk_d = nc.dram_tensor("wk", [E, HL], F16, kind="ExternalInput").ap()
    wv_d = nc.dram_tensor("wv", [E, HL], F16, kind="ExternalInput").ap()
    wo_d = nc.dram_tensor("wo", [HL, E], F16, kind="ExternalInput").ap()
    cos_d = nc.dram_tensor("cosx", [128, S], F32, kind="ExternalInput").ap()
    sin_d = nc.dram_tensor("sinx", [128, S], F32, kind="ExternalInput").ap()
    out_d = nc.dram_tensor("out", [S, E], F32, kind="ExternalOutput").ap()
    zscr_d = nc.dram_tensor("zscr", [HPC, S], F32).ap()  # internal scratch

    ECH = E // 128   # 8 e-chunks
    SCH = S // 128   # 16 seq chunks
    SB = S // 512    # 4 seq blocks
    swap_mask = []
    for i in range(16):
        swap_mask += [2 * i + 1, 2 * i]

    with tile.TileContext(nc) as tc:
        with (
            tc.tile_pool(name="persist", bufs=1) as pp,
            tc.tile_pool(name="evict", bufs=6) as ev,
            tc.tile_pool(name="bswp", bufs=2) as bswp,
        ):
            # persistent tiles
            qT = [pp.tile([128, S], F16, tag=f"qT{c}", name=f"qT{c}") for c in range(2)]
            kT = [pp.tile([128, S], F16, tag=f"kT{c}", name=f"kT{c}") for c in range(2)]
            vt = [pp.tile([128, HPC * (D + 1)], F16, tag=f"v{t}", name=f"v{t}")
                  for t in range(SCH)]
            oT = [pp.tile([128, S], F16, tag=f"oT{c}", name=f"oT{c}") for c in range(2)]
            cosx = pp.tile([128, S], F32R, tag="cosx", name="cosx")
            sinx = pp.tile([128, S], F32R, tag="sinx", name="sinx")
            wo_t = pp.tile([128, 2, E], F16, tag="wo", name="wo")
            wv_t = pp.tile([128, ECH, HL], F16, tag="wv", name="wv")
            xt = [pp.tile([128, S], F16, tag=f"x{e}", name=f"x{e}")
                  for e in range(ECH)]

            def rope(dest, c):
                sw = bswp.tile([128, S], F16, tag="swp", name="swp")
                nc.vector.stream_shuffle(out=sw[:], in_=dest[c][:], mask=swap_mask)
                nc.vector.tensor_mul(out=sw[:], in0=sw[:], in1=sinx[:])
                nc.vector.tensor_mul(out=dest[c][:], in0=dest[c][:], in1=cosx[:])
                nc.vector.tensor_add(out=dest[c][:], in0=dest[c][:], in1=sw[:])

            # ---------------- Phase B: q/k projections + RoPE ----------------
            with (
                tc.tile_pool(name="bw", bufs=1) as bw,
                tc.tile_pool(name="bps", bufs=8, space="PSUM") as bps,
            ):
                wq_t = bw.tile([128, ECH, HL], F16, tag="wq", name="wq")
                wk_t = bw.tile([128, ECH, HL], F16, tag="wk", name="wk")

                def wdma(eng, w_t_, w_d_):
                    eng.dma_start(
                        out=w_t_[:],
                        in_=w_d_.rearrange("(c p) m -> p c m", p=128),
                    )

                def xdma(eng, e, j):
                    eng.dma_start(
                        out=xt[e][:, ts(j, 512)],
                        in_=xT_d[e * 128:(e + 1) * 128, ts(j, 512)],
                    )
                # feed order: j-block 0 slices of every chunk first so the
                # first projection psum group completes within a few us
                wdma(nc.sync, wq_t, wq_d)
                wdma(nc.scalar, wk_t, wk_d)
                for j in range(SB):
                    for e in range(0, ECH, 2):
                        xdma(nc.sync, e, j)
                    for e in range(1, ECH, 2):
                        xdma(nc.scalar, e, j)
                nc.scalar.dma_start(out=cosx[:], in_=cos_d.bitcast(F32R))
                nc.scalar.dma_start(out=sinx[:], in_=sin_d.bitcast(F32R))
                wdma(nc.scalar, wv_t, wv_d)
                nc.scalar.dma_start(
                    out=wo_t[:],
                    in_=wo_d.rearrange("(c p) e -> p c e", p=128),
                )

                for w_t_, dest in ((wq_t, qT), (wk_t, kT)):
                    for m in range(2):
                        for j in range(SB):
                            ps = bps.tile([128, 512], F32, tag="mm", name="mm")
                            for e in range(ECH):
                                nc.tensor.matmul(
                                    ps[:],
                                    w_t_[:, e, m * 128:(m + 1) * 128],
                                    xt[e][:, ts(j, 512)],
                                    start=(e == 0),
                                    stop=(e == ECH - 1),
                                )
                            nc.vector.tensor_copy(
                                out=dest[m][:, ts(j, 512)], in_=ps[:]
                            )
                        if m == 0:
                            rope(dest, m)

            # -------- Phase C: attention with v-proj / Wo-proj interleaved ----
            with (
                tc.tile_pool(name="cexp", bufs=4) as cexp,
                tc.tile_pool(name="cz", bufs=4) as cz,
                tc.tile_pool(name="crb", bufs=2) as crb,
                tc.tile_pool(name="csc", bufs=2, space="PSUM") as csc,
                tc.tile_pool(name="cpv", bufs=1, space="PSUM") as cpv,
                tc.tile_pool(name="cmm", bufs=2, space="PSUM") as cmm,
            ):
                def vgrp(ts0):
                    # v projection for seq chunks [ts0, ts0+4)
                    for t in range(ts0, ts0 + 4):
                        nc.gpsimd.memset(
                            vt[t].rearrange("p (h c) -> p h c", c=D + 1)
                            [:, :, D:D + 1],
                            1.0,
                        )
                        ps = cmm.tile([128, HL], F32, tag="mm", name="mmv")
                        for e in range(ECH):
                            nc.tensor.matmul(
                                ps[:],
                                xt[e][:, ts(t, 128)],
                                wv_t[:, e, :],
                                start=(e == 0),
                                stop=(e == ECH - 1),
                            )
                        nc.vector.tensor_copy(
                            out=vt[t].rearrange("p (h c) -> p h c", c=D + 1)
                            [:, :, 0:D],
                            in_=ps.rearrange("p (h c) -> p h c", c=D),
                        )

                def dgrp(ts0):
                    # Wo projection for seq chunks [ts0, ts0+4)
                    for t in range(ts0, ts0 + 4):
                        for n in range(2):
                            ps = cmm.tile([128, 512], F32, tag="mm", name="wops")
                            for c in range(2):
                                nc.tensor.matmul(
                                    ps[:],
                                    oT[c][:, ts(t, 128)],
                                    wo_t[:, c, ts(n, 512)],
                                    start=(c == 0),
                                    stop=(c == 1),
                                )
                            ot = ev.tile([128, 512], F32, tag="out", name="oev")
                            if (2 * t + n) % 2 == 0:
                                nc.vector.tensor_copy(out=ot[:], in_=ps[:])
                            else:
                                nc.scalar.copy(out=ot[:], in_=ps[:])
                            nc.sync.dma_start(
                                out=out_d[ts(t, 128), ts(n, 512)], in_=ot[:]
                            )

                def attn_block(c, j):
                    hs = [2 * c, 2 * c + 1]
                    qs = [qT[c][0:64, :], qT[c][64:128, :]]
                    ks = [kT[c][0:64, :], kT[c][64:128, :]]
                    nt = 4 * (j + 1)
                    pv = [cpv.tile([65, 512], F32, tag=f"pv{i}", name=f"pv{i}")
                          for i in range(2)]
                    for tp in range(nt // 2):
                        sc = [csc.tile([128, 1024], F32, tag="sc", name="sc")
                              for _ in range(2)]
                        # head 0 on PE rows 0-63, head 1 on rows 64-127:
                        # adjacent pairs overlap in the array
                        for half in range(2):
                            t = 2 * tp + half
                            for i in range(2):
                                nc.tensor.matmul(
                                    sc[i][:, ts(half, 512)],
                                    ks[i][:, ts(t, 128)],
                                    qs[i][:, ts(j, 512)],
                                    start=True,
                                    stop=True,
                                    tile_position=(i * 64, 0),
                                )
                        exm = []
                        for i in range(2):
                            ex = cexp.tile([128, 1024], F16, tag="ex", name="ex")
                            nc.scalar.activation(
                                out=ex[:], in_=sc[i][:], func=Exp, scale=0.125
                            )
                            exm.append(ex)
                        for half in range(2):
                            t = 2 * tp + half
                            if t >= nt - 4:  # diagonal chunk: causal mask
                                for i in range(2):
                                    nc.gpsimd.affine_select(
                                        out=exm[i][:, ts(half, 512)],
                                        in_=exm[i][:, ts(half, 512)],
                                        compare_op=is_ge,
                                        fill=0.0,
                                        base=(j * 512 - t * 128),
                                        channel_multiplier=-1,
                                        pattern=[[1, 512]],
                                    )
                            for i in range(2):
                                vh = vt[t].rearrange(
                                    "p (h c) -> p h c", c=D + 1)[:, hs[i], :]
                                nc.tensor.matmul(
                                    pv[i][:],
                                    vh,
                                    exm[i][:, ts(half, 512)],
                                    start=(t == 0),
                                    stop=(t == nt - 1),
                                )
                    # evict + per-block softmax normalization (overlaps the
                    # following blocks): Z -> DRAM -> partition-broadcast back,
                    # invert on 128 lanes, scale this block of oT in place
                    zq = cz.tile([65, 2, 512], F32, tag="zq", name="zq")
                    for i in range(2):
                        nc.vector.tensor_copy(
                            out=oT[c][i * 64:(i + 1) * 64, ts(j, 512)],
                            in_=pv[i][0:64, :],
                        )
                        nc.vector.tensor_copy(
                            out=zq[64:65, i, :], in_=pv[i][64:65, :]
                        )
                    for i in range(2):
                        nc.sync.dma_start(
                            out=zscr_d[hs[i], ts(j, 512)], in_=zq[64:65, i, :]
                        )
                    rb = crb.tile([128, 512], F32, tag="rb", name="rb")
                    for i in range(2):
                        nc.sync.dma_start(
                            out=rb[i * 64:(i + 1) * 64, :],
                            in_=zscr_d[hs[i]:hs[i] + 1, ts(j, 512)]
                            .to_broadcast((64, 512)),
                        )
                    nc.vector.tensor_tensor(
                        out=oT[c][:, ts(j, 512)],
                        in0=oT[c][:, ts(j, 512)],
                        in1=rb[:],
                        op=mybir.AluOpType.divide,
                    )

                vgrp(0)
                attn_block(0, 0)
                vgrp(4)
                attn_block(0, 1)
                rope(qT, 1)
                rope(kT, 1)
                vgrp(8)
                attn_block(0, 2)
                vgrp(12)
                attn_block(0, 3)
                attn_block(1, 0)
                attn_block(1, 1)
                dgrp(0)
                attn_block(1, 2)
                dgrp(4)
                attn_block(1, 3)
                dgrp(8)
                dgrp(12)

    nc.compile()
    return nc


def _rope_tables():
    iexp = np.arange(0, D, 2, dtype=np.float32) / np.float32(D)
    inv_freq = np.reciprocal(np.power(np.float32(ROPE_BASE), iexp))  # (32,) f32
    ang = np.arange(S, dtype=np.float32)[:, None] * inv_freq[None, :]  # (S, 32)
    cos = np.cos(ang).astype(np.float32)  # (S, 32)
    sin = np.sin(ang).astype(np.float32)
    cosx = np.empty((64, S), dtype=np.float32)
    sinx = np.empty((64, S), dtype=np.float32)
    cosx[0::2] = cos.T
    cosx[1::2] = cos.T
    sinx[0::2] = -sin.T
    sinx[1::2] = sin.T
    return np.tile(cosx, (2, 1)), np.tile(sinx, (2, 1))  # (128, S) each


def get_nc():
    global _built
    if _built is None:
        _built = _build_nc()
    return _built


def make_in_maps(x, Wq, Wk, Wv, Wo):
    cosx, sinx = _rope_tables()
    in_maps = []
    for c in range(NCORES):
        b, g = c // 4, c % 4
        sl = slice(g * HL, (g + 1) * HL)
        in_maps.append({
            "xT": np.ascontiguousarray(x[b].T).astype(np.float16),
            "wq": np.ascontiguousarray(Wq[:, sl]).astype(np.float16),
            "wk": np.ascontiguousarray(Wk[:, sl]).astype(np.float16),
            "wv": np.ascontiguousarray(Wv[:, sl]).astype(np.float16),
            "wo": np.ascontiguousarray(Wo[sl, :]).astype(np.float16),
            "cosx": cosx,
            "sinx": sinx,
        })
    return in_maps


def gather(results):
    out = np.empty((B, S, E), dtype=np.float32)
    for b in range(B):
        acc = results[4 * b]["out"].astype(np.float32).copy()
        for g in range(1, 4):
            acc += results[4 * b + g]["out"]
        out[b] = acc
    return out


def kernel(x, Wq, Wk, Wv, Wo):
    from concourse.bass_utils import run_bass_kernel_spmd

    nc = get_nc()
    in_maps = make_in_maps(
        np.asarray(x), np.asarray(Wq), np.asarray(Wk), np.asarray(Wv), np.asarray(Wo)
    )
    res = run_bass_kernel_spmd(nc, in_maps, list(range(NCORES)))
    return gather(res.results)
